# revision 3
# baseline (speedup 1.0000x reference)
"""Causal single-head attention on 8 trn2 NeuronCores.

Problem: x[4,4096,768], WQ/WK/WV[768,64] -> out[4,4096,64]
  Q=x@WQ K=x@WK V=x@WV; causal softmax(QK^T/8)@V per batch.

Sharding: 2 cores per batch. Causal-balanced query split: even program
handles q-blocks {0,1,6,7} (of 512 rows), odd program {2,3,4,5} -- both
have exactly 72 visible (k-chunk, q-block) pairs. Two SPMD programs run
concurrently on devices [0:4] and [4:8] (async jax dispatch).

Kernel per core (all fp32):
  Phase 1: stream x, PE-transpose to xT, project KT/VT fused ([WK|WV]
    stationary) and QT; build Vplus=[V|1] chunks via PE transpose.
  Phase 2: flash attention with scores TRANSPOSED (keys on partitions):
    scoresT[k,q] = matmul(lhsT=KT chunk, rhs=QT block); exp on ACT
    (scale=1/8, no max subtraction -- scores ~ N(0,1), safe in fp32);
    causal zeroing via gpsimd affine_select on diagonal chunks;
    OT[65,q] += matmul(lhsT=Vplus[128,65], rhs=PT) -- row 64 = softmax
    denominator for free (ones column).
  Epilogue: PE-transpose OT -> O natural, reciprocal * scale, DMA out.
"""
import sys
import os

sys.path.insert(0, "/opt/trn_rl_repo")

import numpy as np

B, S, DM, DK = 4, 4096, 768, 64
NSB = S // 512  # 8 s-blocks of 512 rows
EVEN_BLOCKS = [0, 1, 6, 7]
ODD_BLOCKS = [2, 3, 4, 5]

_cache = {}


def _split_waits(nc, mybir, maxw=1):
    """Walrus here accepts only 1 sem-wait per instruction; move excess
    waits onto preceding same-engine no-ops."""
    cnt = 0
    for bb in nc.m.functions[0].blocks:
        new_insts = []
        for inst in bb.instructions:
            si = inst.sync_info
            if si is not None and si.on_wait and len(si.on_wait) > maxw:
                waits = list(si.on_wait)
                si.on_wait = waits[:maxw]
                extra = waits[maxw:]
                for i in range(0, len(extra), maxw):
                    cnt += 1
                    nop = mybir.InstNoOp(name=f"waitsplit-{cnt}", ins=[], outs=[])
                    nop.engine = inst.engine
                    nop.sync_info = mybir.SyncInfo(
                        on_wait=extra[i : i + maxw], on_update=[]
                    )
                    new_insts.append(nop)
            new_insts.append(inst)
        bb.instructions[:] = new_insts


def _build_program(blocks):
    import concourse.bass as bass
    import concourse.mybir as mybir
    from concourse.tile import TileContext
    from concourse.masks import make_identity

    f32 = mybir.dt.float32
    AF = mybir.ActivationFunctionType

    nc = bass.Bass()
    x = nc.declare_dram_parameter("x", [S, DM], f32, isOutput=False)
    wq = nc.declare_dram_parameter("wq", [DM, DK], f32, isOutput=False)
    wk = nc.declare_dram_parameter("wk", [DM, DK], f32, isOutput=False)
    wv = nc.declare_dram_parameter("wv", [DM, DK], f32, isOutput=False)
    out = nc.declare_dram_parameter("out", [4 * 512, DK], f32, isOutput=True)

    with TileContext(nc) as tc:
        with (
            tc.tile_pool(name="consts", bufs=1) as cpool,
            tc.tile_pool(name="big", bufs=1) as big,
        ):
            ident = cpool.tile([128, 128], f32)
            make_identity(nc, ident[:])
            # [WK|WV] stationary chunks: cols 0:64 = WK, 64:128 = WV
            wkv = cpool.tile([128, 6 * 128], f32)
            wqt = cpool.tile([128, 6 * 64], f32)
            for c in range(6):
                nc.sync.dma_start(
                    wkv[:, c * 128 : c * 128 + 64], wk[c * 128 : (c + 1) * 128, :]
                )
                nc.sync.dma_start(
                    wkv[:, c * 128 + 64 : (c + 1) * 128],
                    wv[c * 128 : (c + 1) * 128, :],
                )
                nc.sync.dma_start(
                    wqt[:, c * 64 : (c + 1) * 64], wq[c * 128 : (c + 1) * 128, :]
                )

            KTVT = big.tile([128, S], f32)  # rows 0:64 = K^T, 64:128 = V^T
            QT = big.tile([64, 4 * 512], f32)  # own queries, transposed
            VP = big.tile([128, 32 * 65], f32)  # [V|1] per k-chunk
            VPr = VP[:].rearrange("p (c u) -> p c u", u=65)
            for kc in range(32):
                nc.gpsimd.memset(VPr[:, kc, 64:65], 1.0)

            # ---------------- Phase 1: projections ----------------
            with (
                tc.tile_pool(name="xload", bufs=2) as xl_pool,
                tc.tile_pool(name="xt", bufs=2) as xt_pool,
                tc.tile_pool(name="tp_ps", bufs=2, space="PSUM") as tp_psum,
                tc.tile_pool(name="kv_ps", bufs=2, space="PSUM") as kv_psum,
                tc.tile_pool(name="qt_ps", bufs=2, space="PSUM") as qt_psum,
            ):
                for sb in range(NSB):
                    xin = xl_pool.tile([128, 4, DM], f32, tag="xin")
                    xv = x[sb * 512 : (sb + 1) * 512, :].rearrange(
                        "(u p) d -> p u d", p=128
                    )
                    nc.sync.dma_start(xin[:], xv)
                    xt = xt_pool.tile([128, 6, 512], f32, tag="xt")
                    for c in range(6):
                        tp = tp_psum.tile([128, 512], f32, tag="tp")
                        for u in range(4):
                            nc.tensor.transpose(
                                tp[:, u * 128 : (u + 1) * 128],
                                xin[:, u, c * 128 : (c + 1) * 128],
                                ident[:],
                            )
                        nc.scalar.copy(xt[:, c, :], tp[:])
                    kv = kv_psum.tile([128, 512], f32, tag="kv")
                    for c in range(6):
                        nc.tensor.matmul(
                            kv[:],
                            wkv[:, c * 128 : (c + 1) * 128],
                            xt[:, c, :],
                            start=(c == 0),
                            stop=(c == 5),
                        )
                    nc.vector.tensor_copy(KTVT[:, sb * 512 : (sb + 1) * 512], kv[:])
                    if sb in blocks:
                        qb = blocks.index(sb)
                        qt = qt_psum.tile([64, 512], f32, tag="qt")
                        for c in range(6):
                            nc.tensor.matmul(
                                qt[:],
                                wqt[:, c * 64 : (c + 1) * 64],
                                xt[:, c, :],
                                start=(c == 0),
                                stop=(c == 5),
                            )
                        nc.vector.tensor_copy(
                            QT[:, qb * 512 : (qb + 1) * 512], qt[:]
                        )
                    # Vplus chunks for this s-block (V natural layout)
                    for u in range(4):
                        kc = sb * 4 + u
                        vtp = tp_psum.tile([128, 64], f32, tag="vtp")
                        nc.tensor.transpose(
                            vtp[:, :],
                            KTVT[64:128, kc * 128 : (kc + 1) * 128],
                            ident[64:128, 64:128],
                        )
                        nc.scalar.copy(VPr[:, kc, 0:64], vtp[:, :])

            # ---------------- Phase 2: attention ----------------
            with (
                tc.tile_pool(name="pt", bufs=6) as pt_pool,
                tc.tile_pool(name="s_ps", bufs=4, space="PSUM") as s_psum,
                tc.tile_pool(name="ot_ps", bufs=1, space="PSUM") as ot_psum,
                tc.tile_pool(name="ep", bufs=2) as ep_pool,
            ):
                ot = [
                    ot_psum.tile([65, 512], f32, name=f"ot{i}", tag=f"ot{i}")
                    for i in range(4)
                ]
                nmax = 4 * blocks[-1] + 4
                for kc in range(nmax):
                    vis = [qb for qb in range(4) if kc < 4 * blocks[qb] + 4]
                    pts = {}
                    for qb in vis:
                        st = s_psum.tile([128, 512], f32, tag="st")
                        nc.tensor.matmul(
                            st[:],
                            KTVT[0:64, kc * 128 : (kc + 1) * 128],
                            QT[:, qb * 512 : (qb + 1) * 512],
                            start=True,
                            stop=True,
                        )
                        pt = pt_pool.tile([128, 512], f32, tag="pt")
                        nc.scalar.activation(pt[:], st[:], AF.Exp, scale=0.125)
                        i = kc - 4 * blocks[qb]
                        if i >= 0:  # diagonal chunk: zero masked region
                            nc.gpsimd.affine_select(
                                out=pt[:],
                                in_=pt[:],
                                compare_op=mybir.AluOpType.is_ge,
                                fill=0.0,
                                base=-128 * i,
                                pattern=[[1, 512]],
                                channel_multiplier=-1,
                            )
                        pts[qb] = pt
                    for qb in vis:
                        nc.tensor.matmul(
                            ot[qb][:],
                            VPr[:, kc, :],
                            pts[qb][:],
                            start=(kc == 0),
                            stop=(kc == 4 * blocks[qb] + 3),
                        )
                # ---------------- Epilogue ----------------
                for qb in range(4):
                    ots = ep_pool.tile([65, 512], f32, tag="ots")
                    nc.scalar.copy(ots[:], ot[qb][:])
                    o_n = ep_pool.tile([128, 4, 65], f32, tag="on")
                    rec = ep_pool.tile([128, 4], f32, tag="rc")
                    for u in range(4):
                        tp2 = s_psum.tile([128, 512], f32, tag="st")
                        nc.tensor.transpose(
                            tp2[:, 0:65],
                            ots[:, u * 128 : (u + 1) * 128],
                            ident[0:65, 0:65],
                        )
                        nc.scalar.copy(o_n[:, u, :], tp2[:, 0:65])
                        nc.vector.reciprocal(rec[:, u : u + 1], o_n[:, u, 64:65])
                        nc.vector.tensor_scalar_mul(
                            o_n[:, u, 0:64], o_n[:, u, 0:64], rec[:, u : u + 1]
                        )
                    ov = out[qb * 512 : (qb + 1) * 512, :].rearrange(
                        "(u p) d -> p u d", p=128
                    )
                    nc.sync.dma_start(ov, o_n[:, :, 0:64])

    _split_waits(nc, mybir)
    return nc


def _make_runner(nc, n_cores, dev_offset):
    """Like bass2jax.run_bass_via_pjrt but with explicit device subset and
    reusable jitted callable."""
    import jax
    import concourse.mybir as mybir
    from concourse import bass2jax
    from jax.experimental.shard_map import shard_map
    from jax.sharding import Mesh, PartitionSpec

    bass2jax.install_neuronx_cc_hook()

    partition_name = (
        nc.partition_id_tensor.name if nc.partition_id_tensor else None
    )
    in_names, out_names, out_avals, zero_outs = [], [], [], []
    for alloc in nc.m.functions[0].allocations:
        if not isinstance(alloc, mybir.MemoryLocationSet):
            continue
        name = alloc.memorylocations[0].name
        if alloc.kind == "ExternalInput":
            if name != partition_name:
                in_names.append(name)
        elif alloc.kind == "ExternalOutput":
            shape = tuple(alloc.tensor_shape)
            dtype = mybir.dt.np(alloc.dtype)
            out_avals.append(jax.core.ShapedArray(shape, dtype))
            out_names.append(name)
            zero_outs.append(np.zeros(shape, dtype))
    n_params = len(in_names)
    n_outs = len(out_avals)
    all_names = in_names + out_names
    if partition_name is not None:
        all_names.append(partition_name)
    donate = tuple(range(n_params, n_params + n_outs))

    def _body(*args):
        operands = list(args)
        if partition_name is not None:
            operands.append(bass2jax.partition_id_tensor())
        outs = bass2jax._bass_exec_p.bind(
            *operands,
            out_avals=tuple(out_avals),
            in_names=tuple(all_names),
            out_names=tuple(out_names),
            lowering_input_output_aliases=(),
            sim_require_finite=True,
            sim_require_nnan=True,
            nc=nc,
        )
        return tuple(outs)

    devices = jax.devices()[dev_offset : dev_offset + n_cores]
    mesh = Mesh(np.asarray(devices), ("core",))
    in_specs = (PartitionSpec("core"),) * (n_params + n_outs)
    out_specs = (PartitionSpec("core"),) * n_outs
    sharded = jax.jit(
        shard_map(
            _body, mesh=mesh, in_specs=in_specs, out_specs=out_specs, check_rep=False
        ),
        donate_argnums=donate,
        keep_unused=True,
    )

    def run(in_maps):
        per_core = [[np.asarray(m[n]) for n in in_names] for m in in_maps]
        concat_in = [
            np.concatenate([per_core[c][i] for c in range(n_cores)], axis=0)
            for i in range(n_params)
        ]
        concat_zeros = [
            np.zeros((n_cores * z.shape[0], *z.shape[1:]), z.dtype)
            for z in zero_outs
        ]
        return sharded(*concat_in, *concat_zeros)

    run.out_names = out_names
    run.out_avals = out_avals
    run.n_cores = n_cores
    return run


def _get_runners():
    if "runners" not in _cache:
        nc_even = _build_program(EVEN_BLOCKS)
        nc_odd = _build_program(ODD_BLOCKS)
        _cache["runners"] = (
            _make_runner(nc_even, 4, 0),
            _make_runner(nc_odd, 4, 4),
        )
    return _cache["runners"]


def kernel(x, WQ, WK, WV):
    import jax

    run_even, run_odd = _get_runners()
    maps = [
        {"x": np.ascontiguousarray(x[b]), "wq": WQ, "wk": WK, "wv": WV}
        for b in range(B)
    ]
    # async dispatch: even program on devices 0-3, odd on 4-7, concurrent
    oa = run_even(maps)
    ob = run_odd(maps)
    ra = np.asarray(oa[0]).reshape(4, 2048, DK)
    rb = np.asarray(ob[0]).reshape(4, 2048, DK)
    out = np.empty((B, S, DK), np.float32)
    for b in range(B):
        for i, J in enumerate(EVEN_BLOCKS):
            out[b, J * 512 : (J + 1) * 512] = ra[b, i * 512 : (i + 1) * 512]
        for i, J in enumerate(ODD_BLOCKS):
            out[b, J * 512 : (J + 1) * 512] = rb[b, i * 512 : (i + 1) * 512]
    return out


if __name__ == "__main__":
    rng = np.random.default_rng(0)
    x = rng.standard_normal((B, S, DM), dtype=np.float32)
    sc = 1.0 / np.sqrt(DM)
    WQ = rng.standard_normal((DM, DK), dtype=np.float32) * sc
    WK = rng.standard_normal((DM, DK), dtype=np.float32) * sc
    WV = rng.standard_normal((DM, DK), dtype=np.float32) * sc
    got = kernel(x, WQ, WK, WV)
    # numpy reference
    Q = x @ WQ
    K = x @ WK
    V = x @ WV
    sref = np.einsum("bqd,bkd->bqk", Q, K) / 8.0
    mask = np.tril(np.ones((S, S), bool))
    sref = np.where(mask, sref, -np.inf)
    sref = sref - sref.max(-1, keepdims=True)
    p = np.exp(sref)
    p /= p.sum(-1, keepdims=True)
    ref = np.einsum("bqk,bkv->bqv", p, V)
    err = np.abs(got - ref).max() / np.abs(ref).max()
    print("rel err:", err)


# revision 4
# speedup vs baseline: 230.1125x; 230.1125x over previous
"""Causal single-head attention on 8 trn2 NeuronCores.

Problem: x[4,4096,768], WQ/WK/WV[768,64] -> out[4,4096,64]
  Q=x@WQ K=x@WK V=x@WV; causal softmax(QK^T/8)@V per batch.

Sharding: 2 cores per batch. Causal-balanced query split: even program
handles q-blocks {0,1,6,7} (of 512 rows), odd program {2,3,4,5} -- both
have exactly 72 visible (k-chunk, q-block) pairs. Two SPMD programs run
concurrently on devices [0:4] and [4:8] (async jax dispatch).

Kernel per core (all fp32):
  Phase 1: stream x, PE-transpose to xT, project KT/VT fused ([WK|WV]
    stationary) and QT; build Vplus=[V|1] chunks via PE transpose.
  Phase 2: flash attention with scores TRANSPOSED (keys on partitions):
    scoresT[k,q] = matmul(lhsT=KT chunk, rhs=QT block); exp on ACT
    (scale=1/8, no max subtraction -- scores ~ N(0,1), safe in fp32);
    causal zeroing via gpsimd affine_select on diagonal chunks;
    OT[65,q] += matmul(lhsT=Vplus[128,65], rhs=PT) -- row 64 = softmax
    denominator for free (ones column).
  Epilogue: PE-transpose OT -> O natural, reciprocal * scale, DMA out.
"""
import sys
import os

sys.path.insert(0, "/opt/trn_rl_repo")

import numpy as np

B, S, DM, DK = 4, 4096, 768, 64
NSB = S // 512  # 8 s-blocks of 512 rows
EVEN_BLOCKS = [0, 1, 6, 7]
ODD_BLOCKS = [2, 3, 4, 5]

_cache = {}


def _split_waits(nc, mybir, maxw=1):
    """Walrus here accepts only 1 sem-wait per instruction; move excess
    waits onto preceding same-engine no-ops."""
    cnt = 0
    for bb in nc.m.functions[0].blocks:
        new_insts = []
        for inst in bb.instructions:
            si = inst.sync_info
            if si is not None and si.on_wait and len(si.on_wait) > maxw:
                waits = list(si.on_wait)
                si.on_wait = waits[:maxw]
                extra = waits[maxw:]
                for i in range(0, len(extra), maxw):
                    cnt += 1
                    nop = mybir.InstNoOp(name=f"waitsplit-{cnt}", ins=[], outs=[])
                    nop.engine = inst.engine
                    nop.sync_info = mybir.SyncInfo(
                        on_wait=extra[i : i + maxw], on_update=[]
                    )
                    new_insts.append(nop)
            new_insts.append(inst)
        bb.instructions[:] = new_insts


def _build_program(blocks):
    import concourse.bass as bass
    import concourse.mybir as mybir
    from concourse.tile import TileContext
    from concourse.masks import make_identity

    f32 = mybir.dt.float32
    AF = mybir.ActivationFunctionType

    nc = bass.Bass()
    x = nc.declare_dram_parameter("x", [S, DM], f32, isOutput=False)
    wq = nc.declare_dram_parameter("wq", [DM, DK], f32, isOutput=False)
    wk = nc.declare_dram_parameter("wk", [DM, DK], f32, isOutput=False)
    wv = nc.declare_dram_parameter("wv", [DM, DK], f32, isOutput=False)
    out = nc.declare_dram_parameter("out", [4 * 512, DK], f32, isOutput=True)

    with TileContext(nc) as tc:
        with (
            tc.tile_pool(name="consts", bufs=1) as cpool,
            tc.tile_pool(name="big", bufs=1) as big,
        ):
            ident = cpool.tile([128, 128], f32)
            make_identity(nc, ident[:])
            # [WK|WV] stationary chunks: cols 0:64 = WK, 64:128 = WV
            wkv = cpool.tile([128, 6 * 128], f32)
            wqt = cpool.tile([128, 6 * 64], f32)
            for c in range(6):
                nc.sync.dma_start(
                    wkv[:, c * 128 : c * 128 + 64], wk[c * 128 : (c + 1) * 128, :]
                )
                nc.sync.dma_start(
                    wkv[:, c * 128 + 64 : (c + 1) * 128],
                    wv[c * 128 : (c + 1) * 128, :],
                )
                nc.sync.dma_start(
                    wqt[:, c * 64 : (c + 1) * 64], wq[c * 128 : (c + 1) * 128, :]
                )

            KTVT = big.tile([128, S], f32)  # rows 0:64 = K^T, 64:128 = V^T
            QT = big.tile([64, 4 * 512], f32)  # own queries, transposed
            VP = big.tile([128, 32 * 65], f32)  # [V|1] per k-chunk
            VPr = VP[:].rearrange("p (c u) -> p c u", u=65)
            for kc in range(32):
                nc.gpsimd.memset(VPr[:, kc, 64:65], 1.0)

            # ---------------- Phase 1: projections ----------------
            with (
                tc.tile_pool(name="xload", bufs=2) as xl_pool,
                tc.tile_pool(name="xt", bufs=2) as xt_pool,
                tc.tile_pool(name="tp_ps", bufs=2, space="PSUM") as tp_psum,
                tc.tile_pool(name="kv_ps", bufs=2, space="PSUM") as kv_psum,
                tc.tile_pool(name="qt_ps", bufs=2, space="PSUM") as qt_psum,
            ):
                for sb in range(NSB):
                    xin = xl_pool.tile([128, 4, DM], f32, tag="xin")
                    xv = x[sb * 512 : (sb + 1) * 512, :].rearrange(
                        "(u p) d -> p u d", p=128
                    )
                    nc.sync.dma_start(xin[:], xv)
                    xt = xt_pool.tile([128, 6, 512], f32, tag="xt")
                    for c in range(6):
                        tp = tp_psum.tile([128, 512], f32, tag="tp")
                        for u in range(4):
                            nc.tensor.transpose(
                                tp[:, u * 128 : (u + 1) * 128],
                                xin[:, u, c * 128 : (c + 1) * 128],
                                ident[:],
                            )
                        nc.scalar.copy(xt[:, c, :], tp[:])
                    kv = kv_psum.tile([128, 512], f32, tag="kv")
                    for c in range(6):
                        nc.tensor.matmul(
                            kv[:],
                            wkv[:, c * 128 : (c + 1) * 128],
                            xt[:, c, :],
                            start=(c == 0),
                            stop=(c == 5),
                        )
                    nc.vector.tensor_copy(KTVT[:, sb * 512 : (sb + 1) * 512], kv[:])
                    if sb in blocks:
                        qb = blocks.index(sb)
                        qt = qt_psum.tile([64, 512], f32, tag="qt")
                        for c in range(6):
                            nc.tensor.matmul(
                                qt[:],
                                wqt[:, c * 64 : (c + 1) * 64],
                                xt[:, c, :],
                                start=(c == 0),
                                stop=(c == 5),
                            )
                        nc.vector.tensor_copy(
                            QT[:, qb * 512 : (qb + 1) * 512], qt[:]
                        )
                    # Vplus chunks for this s-block (V natural layout)
                    for u in range(4):
                        kc = sb * 4 + u
                        vtp = tp_psum.tile([128, 64], f32, tag="vtp")
                        nc.tensor.transpose(
                            vtp[:, :],
                            KTVT[64:128, kc * 128 : (kc + 1) * 128],
                            ident[64:128, 64:128],
                        )
                        nc.scalar.copy(VPr[:, kc, 0:64], vtp[:, :])

            # ---------------- Phase 2: attention ----------------
            with (
                tc.tile_pool(name="pt", bufs=6) as pt_pool,
                tc.tile_pool(name="s_ps", bufs=4, space="PSUM") as s_psum,
                tc.tile_pool(name="ot_ps", bufs=1, space="PSUM") as ot_psum,
                tc.tile_pool(name="ep", bufs=2) as ep_pool,
            ):
                ot = [
                    ot_psum.tile([65, 512], f32, name=f"ot{i}", tag=f"ot{i}")
                    for i in range(4)
                ]
                nmax = 4 * blocks[-1] + 4
                for kc in range(nmax):
                    vis = [qb for qb in range(4) if kc < 4 * blocks[qb] + 4]
                    pts = {}
                    for qb in vis:
                        st = s_psum.tile([128, 512], f32, tag="st")
                        nc.tensor.matmul(
                            st[:],
                            KTVT[0:64, kc * 128 : (kc + 1) * 128],
                            QT[:, qb * 512 : (qb + 1) * 512],
                            start=True,
                            stop=True,
                        )
                        pt = pt_pool.tile([128, 512], f32, tag="pt")
                        nc.scalar.activation(pt[:], st[:], AF.Exp, scale=0.125)
                        i = kc - 4 * blocks[qb]
                        if i >= 0:  # diagonal chunk: zero masked region
                            nc.gpsimd.affine_select(
                                out=pt[:],
                                in_=pt[:],
                                compare_op=mybir.AluOpType.is_ge,
                                fill=0.0,
                                base=-128 * i,
                                pattern=[[1, 512]],
                                channel_multiplier=-1,
                            )
                        pts[qb] = pt
                    for qb in vis:
                        nc.tensor.matmul(
                            ot[qb][:],
                            VPr[:, kc, :],
                            pts[qb][:],
                            start=(kc == 0),
                            stop=(kc == 4 * blocks[qb] + 3),
                        )
                # ---------------- Epilogue ----------------
                for qb in range(4):
                    ots = ep_pool.tile([65, 512], f32, tag="ots")
                    nc.scalar.copy(ots[:], ot[qb][:])
                    o_n = ep_pool.tile([128, 4, 65], f32, tag="on")
                    rec = ep_pool.tile([128, 4], f32, tag="rc")
                    for u in range(4):
                        tp2 = s_psum.tile([128, 512], f32, tag="st")
                        nc.tensor.transpose(
                            tp2[:, 0:65],
                            ots[:, u * 128 : (u + 1) * 128],
                            ident[0:65, 0:65],
                        )
                        nc.scalar.copy(o_n[:, u, :], tp2[:, 0:65])
                        nc.vector.reciprocal(rec[:, u : u + 1], o_n[:, u, 64:65])
                        nc.vector.tensor_scalar_mul(
                            o_n[:, u, 0:64], o_n[:, u, 0:64], rec[:, u : u + 1]
                        )
                    ov = out[qb * 512 : (qb + 1) * 512, :].rearrange(
                        "(u p) d -> p u d", p=128
                    )
                    nc.sync.dma_start(ov, o_n[:, :, 0:64])

    _split_waits(nc, mybir)
    return nc


def _make_runner(nc, n_cores, dev_offset):
    """Like bass2jax.run_bass_via_pjrt but with explicit device subset and
    reusable jitted callable."""
    import jax
    import concourse.mybir as mybir
    from concourse import bass2jax
    from jax.experimental.shard_map import shard_map
    from jax.sharding import Mesh, PartitionSpec

    bass2jax.install_neuronx_cc_hook()

    partition_name = (
        nc.partition_id_tensor.name if nc.partition_id_tensor else None
    )
    in_names, out_names, out_avals, zero_outs = [], [], [], []
    for alloc in nc.m.functions[0].allocations:
        if not isinstance(alloc, mybir.MemoryLocationSet):
            continue
        name = alloc.memorylocations[0].name
        if alloc.kind == "ExternalInput":
            if name != partition_name:
                in_names.append(name)
        elif alloc.kind == "ExternalOutput":
            shape = tuple(alloc.tensor_shape)
            dtype = mybir.dt.np(alloc.dtype)
            out_avals.append(jax.core.ShapedArray(shape, dtype))
            out_names.append(name)
            zero_outs.append(np.zeros(shape, dtype))
    n_params = len(in_names)
    n_outs = len(out_avals)
    all_names = in_names + out_names
    if partition_name is not None:
        all_names.append(partition_name)
    donate = tuple(range(n_params, n_params + n_outs))

    def _body(*args):
        operands = list(args)
        if partition_name is not None:
            operands.append(bass2jax.partition_id_tensor())
        outs = bass2jax._bass_exec_p.bind(
            *operands,
            out_avals=tuple(out_avals),
            in_names=tuple(all_names),
            out_names=tuple(out_names),
            lowering_input_output_aliases=(),
            sim_require_finite=True,
            sim_require_nnan=True,
            nc=nc,
        )
        return tuple(outs)

    devices = jax.devices()[dev_offset : dev_offset + n_cores]
    mesh = Mesh(np.asarray(devices), ("core",))
    in_specs = (PartitionSpec("core"),) * (n_params + n_outs)
    out_specs = (PartitionSpec("core"),) * n_outs
    sharded = jax.jit(
        shard_map(
            _body, mesh=mesh, in_specs=in_specs, out_specs=out_specs, check_rep=False
        ),
        keep_unused=True,
    )
    from jax.sharding import NamedSharding

    sh = NamedSharding(mesh, PartitionSpec("core"))

    def prepare(in_maps):
        per_core = [[np.asarray(m[n]) for n in in_names] for m in in_maps]
        concat_in = [
            np.concatenate([per_core[c][i] for c in range(n_cores)], axis=0)
            for i in range(n_params)
        ]
        concat_zeros = [
            np.zeros((n_cores * z.shape[0], *z.shape[1:]), z.dtype)
            for z in zero_outs
        ]
        return [jax.device_put(a, sh) for a in concat_in + concat_zeros]

    def run(in_maps):
        return sharded(*prepare(in_maps))

    run.sharded = sharded
    run.prepare = prepare
    run.out_names = out_names
    run.out_avals = out_avals
    run.n_cores = n_cores
    return run


def _get_runners():
    if "runners" not in _cache:
        nc_even = _build_program(EVEN_BLOCKS)
        nc_odd = _build_program(ODD_BLOCKS)
        _cache["runners"] = (
            _make_runner(nc_even, 4, 0),
            _make_runner(nc_odd, 4, 4),
        )
    return _cache["runners"]


def kernel(x, WQ, WK, WV):
    import jax

    run_even, run_odd = _get_runners()
    maps = [
        {"x": np.ascontiguousarray(x[b]), "wq": WQ, "wk": WK, "wv": WV}
        for b in range(B)
    ]
    # async dispatch: even program on devices 0-3, odd on 4-7, concurrent
    oa = run_even(maps)
    ob = run_odd(maps)
    ra = np.asarray(oa[0]).reshape(4, 2048, DK)
    rb = np.asarray(ob[0]).reshape(4, 2048, DK)
    out = np.empty((B, S, DK), np.float32)
    for b in range(B):
        for i, J in enumerate(EVEN_BLOCKS):
            out[b, J * 512 : (J + 1) * 512] = ra[b, i * 512 : (i + 1) * 512]
        for i, J in enumerate(ODD_BLOCKS):
            out[b, J * 512 : (J + 1) * 512] = rb[b, i * 512 : (i + 1) * 512]
    return out


if __name__ == "__main__":
    rng = np.random.default_rng(0)
    x = rng.standard_normal((B, S, DM), dtype=np.float32)
    sc = 1.0 / np.sqrt(DM)
    WQ = rng.standard_normal((DM, DK), dtype=np.float32) * sc
    WK = rng.standard_normal((DM, DK), dtype=np.float32) * sc
    WV = rng.standard_normal((DM, DK), dtype=np.float32) * sc
    got = kernel(x, WQ, WK, WV)
    # numpy reference
    Q = x @ WQ
    K = x @ WK
    V = x @ WV
    sref = np.einsum("bqd,bkd->bqk", Q, K) / 8.0
    mask = np.tril(np.ones((S, S), bool))
    sref = np.where(mask, sref, -np.inf)
    sref = sref - sref.max(-1, keepdims=True)
    p = np.exp(sref)
    p /= p.sum(-1, keepdims=True)
    ref = np.einsum("bqk,bkv->bqv", p, V)
    err = np.abs(got - ref).max() / np.abs(ref).max()
    print("rel err:", err)
